# revision 10
# baseline (speedup 1.0000x reference)
"""Trainium2 Bass kernel for nn_Conv2dGeneral (capsule-style 4x4-pose conv).

Math (from the reference):
  out[b,o,X,Y,u,w] = sum_{cin,kx,ky,v} Wm[(cin,kx,ky),o,u,v] * x[b,cin,2X+kx,2Y+ky,4v+w] + bias[o]

Mapped to the PE array as a single 1152-deep contraction:
  K = (cin, v)  x  9 accumulation chunks over (kx, ky)   [9 x 128]
  M = (o, u)                                              [128 PSUM partitions]
  N = (X, Y, w)                                           [676 per batch image]

Data-parallel across 8 NeuronCores on the batch dim (8 images per core).

Host-side prep: x is re-laid-out to [(b), (cin,v), (r,c,w)] so each core's
shard DMAs as fully-contiguous 12.5KB partition lines; the stride-2 im2col
window gather then happens for free inside the matmul moving-operand access
pattern (no patch materialization, each x byte read once from HBM).
"""

import numpy as np

B, CIN, COUT = 64, 32, 32
KK, STRIDE = 3, 2
WIN, HH = 28, 16
H = 4
WOUT = (WIN - KK) // STRIDE + 1  # 13
NCORES = 8
BPC = B // NCORES                # batches per core
RCW = WIN * WIN * H              # 3136 free elements per (cin,v) partition
NOUT = WOUT * WOUT * H           # 676 outputs per (o,u) partition per image
XSPLIT = ((0, 7), (7, 6))        # two PSUM groups: X rows [0,7) and [7,13)

_cache = {}


def _build_bass():
    """Raw-bass build (no Tile): this toolchain's walrus codegen allows only
    ONE sync-wait per instruction, so all cross-engine sync is explicit
    single-sem waits; ordering beyond that rides on hardware transitivity.

    Engines: SP triggers the 7 DMAs, PE runs 16 accumulation groups of 9
    matmuls (one per kernel tap), ACT evicts PSUM->SBUF adding the bias.
    """
    import concourse.bass as bass
    import concourse.mybir as mybir

    f32 = mybir.dt.float32
    GB = 2                    # batches per input-DMA chunk
    OB = 4                    # batches per output-DMA chunk
    NCH = BPC // GB           # 4 input chunks
    NOC = BPC // OB           # 2 output chunks
    NG = 2 * BPC              # 16 PSUM accumulation groups
    GPO = 2 * OB              # groups per output chunk

    nc = bass.Bass()
    x_d = nc.declare_dram_parameter("x", [NCH, 128, GB * RCW], f32, isOutput=False)
    w_d = nc.declare_dram_parameter("w", [128, 9 * 128 + 1], f32, isOutput=False)
    o_d = nc.declare_dram_parameter("out", [NOC, 128, OB * NOUT], f32, isOutput=True)

    with (
        nc.sbuf_tensor([128, 9 * 128 + 1], f32) as wt,
        nc.sbuf_tensor([128, NCH, GB * RCW], f32) as gt,
        nc.sbuf_tensor([128, NOC, OB * NOUT], f32) as ot,
        nc.psum_tensor([128, 8, 512], f32) as ps,
        nc.semaphore("wt_sem") as wt_sem,
        nc.semaphore("g_sem0") as g_sem0,
        nc.semaphore("g_sem1") as g_sem1,
        nc.semaphore("g_sem2") as g_sem2,
        nc.semaphore("g_sem3") as g_sem3,
        nc.semaphore("pe_sem") as pe_sem,
        nc.semaphore("act_sem") as act_sem,
        nc.semaphore("out_sem") as out_sem,
        nc.Block() as block,
    ):
        g_sems = [g_sem0, g_sem1, g_sem2, g_sem3]
        wtr = wt[:, : 9 * 128].rearrange("p (k m) -> p k m", k=9)
        bt = wt[:, 9 * 128 :]

        @block.sync
        def _(sync):
            sync.dma_start(wt[:, :], w_d[:, :]).then_inc(wt_sem, 16)
            for c in range(NCH):
                sync.dma_start(gt[:, c, :], x_d[c]).then_inc(g_sems[c], 16)
            for oc in range(NOC):
                sync.wait_ge(act_sem, GPO * (oc + 1))
                sync.dma_start(o_d[oc], ot[:, oc, :]).then_inc(out_sem, 16)
            sync.wait_ge(out_sem, 16 * NOC)

        @block.tensor
        def _(tensor):
            tensor.wait_ge(wt_sem, 16)
            for j in range(NG):
                b, half = divmod(j, 2)
                c, bi = divmod(b, GB)
                if j % (2 * GB) == 0:
                    tensor.wait_ge(g_sems[c], 16)
                if j >= 8:
                    # PSUM bank j%8 is free once ACT drained group j-8
                    tensor.wait_ge(act_sem, j - 7)
                X0, nX = XSPLIT[half]
                gr = gt[:, c, bi * RCW : (bi + 1) * RCW].rearrange(
                    "p (r c w) -> p r c w", r=WIN, c=WIN
                )
                for kk in range(9):
                    kx, ky = divmod(kk, 3)
                    rhs = gr[
                        :,
                        2 * X0 + kx : 2 * X0 + kx + 2 * nX - 1 : 2,
                        ky : ky + 2 * WOUT - 1 : 2,
                        :,
                    ]
                    mm = tensor.matmul(
                        ps[:, j % 8, : nX * WOUT * H],
                        wtr[:, kk, :],
                        rhs,
                        start=(kk == 0),
                        stop=(kk == 8),
                    )
                mm.then_inc(pe_sem, 1)

        @block.scalar
        def _(scalar):
            for j in range(NG):
                b, half = divmod(j, 2)
                X0, nX = XSPLIT[half]
                oc, obi = divmod(b, OB)
                off = obi * NOUT + X0 * WOUT * H
                scalar.wait_ge(pe_sem, j + 1)
                scalar.activation(
                    ot[:, oc, off : off + nX * WOUT * H],
                    ps[:, j % 8, : nX * WOUT * H],
                    mybir.ActivationFunctionType.Identity,
                    bias=bt[:, :],
                ).then_inc(act_sem, 1)

    return nc


def _prep_inputs(x, W, bias):
    # x: (B, CIN, 28, 28, 16) -> xp[b, cin*4+v, (r*28+c)*4+w] = x[b,cin,r,c,4v+w]
    xp = np.ascontiguousarray(
        x.reshape(B, CIN, WIN, WIN, H, H).transpose(0, 1, 4, 2, 3, 5)
    ).reshape(B, CIN * H, RCW)
    # W: (1, 288, 32, 1, 1, 4, 4); p = cin*9 + kx*3 + ky
    # wt_sb[cin*4+v, kk*128 + o*4+u] = Wm[cin*9+kk, o, u, v]
    Wm = np.asarray(W, dtype=np.float32).reshape(CIN, KK * KK, COUT, H, H)
    wt_sb = np.ascontiguousarray(
        Wm.transpose(0, 4, 1, 2, 3)  # cin, v, kk, o, u
    ).reshape(128, 9 * 128)
    bias_v = np.repeat(np.asarray(bias, dtype=np.float32).reshape(COUT), H).reshape(
        128, 1
    )
    wtb = np.ascontiguousarray(np.concatenate([wt_sb, bias_v], axis=1))
    return xp.astype(np.float32, copy=False), wtb


def _shard_x(xp, core, gb=2):
    # per-core chunked input: [BPC//gb, 128, gb*RCW], chunk c = batches (c*gb..)
    s = xp[core * BPC : (core + 1) * BPC]
    return np.ascontiguousarray(
        s.reshape(BPC // gb, gb, 128, RCW).transpose(0, 2, 1, 3)
    ).reshape(BPC // gb, 128, gb * RCW)


def _unchunk_out(dev_out, ob=4):
    # dev_out: (BPC//ob, 128, ob*NOUT) -> (BPC, 128, NOUT)
    return (
        dev_out.reshape(BPC // ob, 128, ob, NOUT)
        .transpose(0, 2, 1, 3)
        .reshape(BPC, 128, NOUT)
    )


def _unprep_output(full):
    # full: (B, 128, NOUT) with partition o*4+u, free (X, Y, w)
    out = (
        full.reshape(B, COUT, H, WOUT, WOUT, H)
        .transpose(0, 1, 3, 4, 2, 5)
        .reshape(B, COUT, WOUT, WOUT, HH)
    )
    return np.ascontiguousarray(out)


def run_device(in_maps, trace=False, tmpdir=None):
    from concourse.bass_utils import run_bass_kernel_spmd

    if "nc" not in _cache:
        _cache["nc"] = _build_bass()
    return run_bass_kernel_spmd(
        _cache["nc"], in_maps, list(range(NCORES)), trace=trace, tmpdir=tmpdir
    )


def kernel(x, W, bias):
    x = np.asarray(x, dtype=np.float32)
    xp, wtb = _prep_inputs(x, W, bias)
    in_maps = [{"x": _shard_x(xp, i), "w": wtb} for i in range(NCORES)]
    res = run_device(in_maps, trace=False)
    full = np.concatenate(
        [_unchunk_out(res.results[i]["out"]) for i in range(NCORES)], axis=0
    )
    return _unprep_output(full)
